# revision 19
# baseline (speedup 1.0000x reference)
"""DIN activation unit kernel for 8x TRN2 NeuronCores.

Math (per batch row b, per key position t):
  h[t]  = (Wk-Wc) @ k[t] + Wd @ (q*k[t]) + (Wq+Wc) @ q + b1
        = W_b @ k[t] + bias_b,   W_b = (Wk-Wc).T + Wd.T * q  (column-folded)
  s[t]  = w2 . PReLU(h[t], 0.25)
  p     = softmax over masked t;  w = p*mask / max(sum, 1e-6)
  out   = sum_t w[t] * k[t]

Device pipeline (pure data-parallel over batch, 256 rows/core):
  - keys shipped ONCE as bf16 in natural [t, d] row layout (8-row groups,
    LBSEQ-permuted batch order) and transposed to [d, t] during the
    HBM->SBUF load via the XBAR transpose DMA.
  - Per row: W_b = (wd_t * q_col) + wa_t in one DVE scalar_tensor_tensor
    (per-partition fp32 scalar), then a single PE matmul W_b.T @ kT into
    PSUM; Scalar engine applies bias+PReLU; scores via zero-padded-w2
    matmuls accumulating a [128b x 200t] block in PSUM (col-group cycling).
  - Softmax per block on DVE/Scalar; weights normalized (p * 1/den) on the
    Scalar engine before the reduction.
  - Final reduction reuses the SBUF-resident kT tiles: each row's weight
    vector is broadcast across partitions by a PE matmul whose stationary
    is a stride-0 broadcast AP of a one-hot eye column ([64,1] -> [64,128]),
    then one DVE scalar_tensor_tensor multiplies kT by the broadcast
    weights and accumulates along t into an output column (accum_out).
  - Output is [D, 128] per block (d on partitions); host transposes back.
"""

import numpy as np
import ml_dtypes

B, T, D = 2048, 200, 128
NCORES = 8
BC = B // NCORES          # 256 batch rows per core
NBLK = BC // 128          # 2 blocks of 128 rows
NGRP = BC // 8            # 32 groups of 8 rows
BF16 = ml_dtypes.bfloat16
BIG = 1024.0              # mask shift; exp(-~1024) == 0 in fp32

# processing order within a block: cycle the four 32-row PE column groups so
# consecutive scores matmuls run concurrently in distinct col-groups
LBSEQ = [(i % 4) * 32 + i // 4 for i in range(128)]

_CACHE = {}


def _build_module():
    from contextlib import ExitStack

    import concourse.bacc as bacc
    import concourse.mybir as mybir
    from concourse import tile
    from concourse.bass import broadcast_tensor_aps

    fp32 = mybir.dt.float32
    bf16 = mybir.dt.bfloat16
    Alu = mybir.AluOpType
    AF = mybir.ActivationFunctionType

    nc = bacc.Bacc(
        "TRN2", target_bir_lowering=False, debug=False, num_devices=NCORES
    )

    # keys, natural [t, d] layout per row, rows LBSEQ-permuted, 8-row groups
    k8_d = nc.dram_tensor("k8", [NGRP, 8 * T, D], bf16, kind="ExternalInput")
    mf_d = nc.dram_tensor("mf", [BC, T], bf16, kind="ExternalInput")
    qt_d = nc.dram_tensor("qt", [NBLK, D, 128], bf16, kind="ExternalInput")
    bt_d = nc.dram_tensor("bt", [NBLK, D, 128], bf16, kind="ExternalInput")
    wa_d = nc.dram_tensor("wa", [D, D], bf16, kind="ExternalInput")
    wd_d = nc.dram_tensor("wd", [D, D], bf16, kind="ExternalInput")
    w2c_d = nc.dram_tensor("w2c", [D, 1], bf16, kind="ExternalInput")
    eye_d = nc.dram_tensor("eye", [128, 64], bf16, kind="ExternalInput")
    out_d = nc.dram_tensor("out", [NBLK, D, 128], fp32, kind="ExternalOutput")

    k8 = k8_d.ap()
    mf = mf_d.ap()
    qt = qt_d.ap()
    bt = bt_d.ap()
    out = out_d.ap()

    with ExitStack() as ctx:
        tc = ctx.enter_context(tile.TileContext(nc))
        const = ctx.enter_context(tc.tile_pool(name="const", bufs=1))
        ktp = ctx.enter_context(tc.tile_pool(name="ktp", bufs=32))
        wbp = ctx.enter_context(tc.tile_pool(name="wbp", bufs=6))
        hap = ctx.enter_context(tc.tile_pool(name="hap", bufs=12))
        blkp = ctx.enter_context(tc.tile_pool(name="blkp", bufs=2))
        smallp = ctx.enter_context(tc.tile_pool(name="smallp", bufs=4))
        junkp = ctx.enter_context(tc.tile_pool(name="junkp", bufs=4))
        hpp = ctx.enter_context(tc.tile_pool(name="hpp", bufs=4, space="PSUM"))
        pbp = ctx.enter_context(tc.tile_pool(name="pbp", bufs=3, space="PSUM"))
        spp = ctx.enter_context(tc.tile_pool(name="spp", bufs=1, space="PSUM"))

        zw_t = const.tile([D, D], bf16, name="zw_t")
        nc.gpsimd.memset(zw_t[:], 0.0)
        zr_t = const.tile([D, 2 * T], bf16, name="zr_t")
        nc.gpsimd.memset(zr_t[:], 0.0)
        wa_t = const.tile([D, D], bf16, name="wa_t")
        nc.sync.dma_start(wa_t[:], wa_d.ap()[:])
        wd_t = const.tile([D, D], bf16, name="wd_t")
        nc.sync.dma_start(wd_t[:], wd_d.ap()[:])
        # double-diag w2 built on-device: w2q[:, c, c] = w2q[:, c, 32+c] = w2.
        # One score matmul then serves the row pair (lb, lb+32): stationary
        # w2q[:, c, :] spans two adjacent 32-col groups; each row's scores
        # land in its own 200-col slot of the 512-wide score bank.
        w2c_t = const.tile([D, 1], bf16, name="w2c_t")
        nc.sync.dma_start(w2c_t[:], w2c_d.ap()[:])
        w2q_t = const.tile([D, 32, 64], bf16, name="w2q_t")
        nc.gpsimd.memset(w2q_t[:], 0.0)
        for c in range(32):
            nc.vector.tensor_scalar_mul(w2q_t[:, c, c : c + 1], w2c_t[:], 1.0)
            nc.vector.tensor_scalar_mul(w2q_t[:, c, 32 + c : 33 + c], w2c_t[:], 1.0)
        eye_t = const.tile([128, 64], bf16, name="eye_t")
        nc.sync.dma_start(eye_t[:], eye_d.ap()[:])
        # shape reference for broadcasting eye columns to [64, 128] stationary
        dummy64 = const.tile([64, 128], bf16, name="dummy64")
        nc.gpsimd.memset(dummy64[:], 0.0)
        # output accumulators [d, row] per block
        vt2 = [const.tile([D, 128], fp32, name=f"vt2_{b}") for b in range(NBLK)]

        # per-block tiles that live through both phases
        qt_s, bt_s, mf_s, s_ps = [], [], [], []
        for blk in range(NBLK):
            qb = blkp.tile([D, 128], bf16, name="qt_b", tag="qt_b")
            nc.sync.dma_start(qb[:], qt[blk])
            # stt per-partition scalars must be fp32: upconvert on device
            qs = blkp.tile([D, 128], fp32, name="qt_s", tag="qt_s")
            nc.scalar.copy(qs[:], qb[:])
            qt_s.append(qs)
            bs = blkp.tile([D, 128], bf16, name="bt_s", tag="bt_s")
            nc.sync.dma_start(bs[:], bt[blk])
            bt_s.append(bs)
            ms = blkp.tile([128, T], bf16, name="mf_s", tag="mf_s")
            nc.sync.dma_start(ms[:], mf[blk * 128 : (blk + 1) * 128, :])
            mf_s.append(ms)
            # full-bank tile so partition stride is bank-aligned
            sp = spp.tile([128, 512], fp32, name="s_ps", tag="s_ps")
            # zero-weight matmul: zeroes the region and sets every element's
            # has_written bit so all scores matmuls can accumulate in any
            # col-group order
            nc.tensor.matmul(sp[:, 0 : 2 * T], zw_t[:], zr_t[:],
                             start=True, stop=False, skip_group_check=True)
            s_ps.append(sp)

        kts = [[None] * 16 for _ in range(NBLK)]

        def mlp_phase(blk, extra=None):
            for g16 in range(16):
                if extra is not None:
                    extra(g16)
                grp = blk * 16 + g16
                # XBAR transpose load: [8T, D] -> [D, 8, T]
                kt_t = ktp.tile([D, 8, T], bf16, name="kt_t", tag="kt")
                nc.sync.dma_start_transpose(kt_t[:], k8[grp])
                kts[blk][g16] = kt_t
                for i2 in range(4):
                    ha2 = hap.tile([128, 2, T], bf16, name="ha2", tag="ha2")
                    for sslot in range(2):
                        i = 2 * i2 + sslot
                        pos = g16 * 8 + i
                        # fold q into the weights: W_b = wd_t * q_col + wa_t
                        wb = wbp.tile([D, D], bf16, name="wb", tag="wb")
                        nc.vector.scalar_tensor_tensor(
                            wb[:], wd_t[:], qt_s[blk][:, pos : pos + 1], wa_t[:],
                            op0=Alu.mult, op1=Alu.add,
                        )
                        hp = hpp.tile([128, T], fp32, name="hp", tag="hp")
                        nc.tensor.matmul(hp[:], wb[:], kt_t[:, i, :],
                                         start=True, stop=True)
                        nc.scalar.activation(
                            ha2[:, sslot, :], hp[:], AF.Prelu,
                            bias=bt_s[blk][:, pos : pos + 1], scale=1.0,
                            alpha=0.25,
                        )
                    pos0 = g16 * 8 + 2 * i2
                    r1 = LBSEQ[pos0]            # LBSEQ[pos0+1] == r1 + 32
                    g1, c = r1 // 32, r1 % 32
                    nc.tensor.matmul(
                        s_ps[blk][32 * g1 : 32 * g1 + 64, 0 : 2 * T],
                        w2q_t[:, c, :], ha2[:],
                        tile_position=(0, 32 * g1),
                        start=False, stop=(pos0 == 126),
                        skip_group_check=True,
                    )

        def softmax_part(blk):
            # slot s holds scores of rows with (lb//32) % 2 == s at free
            # [s*T : (s+1)*T]; run the softmax once per slot (each partition's
            # result is only valid for its own slot's rows).
            pns = []
            for sslot in range(2):
                sps = s_ps[blk][:, sslot * T : (sslot + 1) * T]
                smt = blkp.tile([128, T], fp32, name="smt", tag=f"smt{sslot}")
                nc.vector.scalar_tensor_tensor(
                    smt[:], sps, BIG, mf_s[blk][:], op0=Alu.add, op1=Alu.mult
                )
                mx = smallp.tile([128, 1], fp32, name="mx", tag=f"mx{sslot}")
                nc.vector.tensor_reduce(
                    mx[:], smt[:], mybir.AxisListType.X, Alu.max)
                nmx = smallp.tile([128, 1], fp32, name="nmx", tag=f"nmx{sslot}")
                nc.vector.tensor_scalar_mul(nmx[:], mx[:], -1.0)
                expv = blkp.tile([128, T], fp32, name="expv", tag=f"expv{sslot}")
                nc.scalar.activation(expv[:], smt[:], AF.Exp, bias=nmx[:])
                p_t = blkp.tile([128, T], bf16, name="p_t", tag=f"p_t{sslot}")
                den = smallp.tile([128, 1], fp32, name="den", tag=f"den{sslot}")
                nc.vector.scalar_tensor_tensor(
                    p_t[:], expv[:], 0.0, mf_s[blk][:],
                    op0=Alu.bypass, op1=Alu.mult, accum_out=den[:],
                )
                denc = smallp.tile([128, 1], fp32, name="denc", tag=f"denc{sslot}")
                nc.vector.tensor_scalar_max(denc[:], den[:], 1e-6)
                rec = smallp.tile([128, 1], fp32, name="rec", tag=f"rec{sslot}")
                nc.vector.reciprocal(rec[:], denc[:])
                # normalized weights (Scalar engine: Copy, per-partition scale)
                pn = blkp.tile([128, T], bf16, name="pn", tag=f"pn{sslot}")
                nc.scalar.mul(pn[:], p_t[:], rec[:])
                pns.append(pn)
            return pns

        def reduce_rows(blk, pns, poss):
            for pos in poss:
                lb = LBSEQ[pos]
                pn = pns[(lb // 32) % 2]
                base = 0 if lb < 64 else 64
                pb = pbp.tile([128, T], fp32, name="pb", tag="pb")
                ecol = eye_t[base : base + 64, lb - base : lb - base + 1]
                eb, _ = broadcast_tensor_aps(ecol, dummy64[:])
                nc.tensor.matmul(pb[:], eb, pn[base : base + 64, :],
                                 start=True, stop=True)
                junk = junkp.tile([128, T], bf16, name="junk", tag="junk")
                nc.vector.scalar_tensor_tensor(
                    junk[:], kts[blk][pos // 8][:, pos % 8, :], 0.0, pb[:],
                    op0=Alu.bypass, op1=Alu.mult,
                    accum_out=vt2[blk][:, lb : lb + 1],
                )

        def out_part(blk):
            nc.sync.dma_start(out[blk], vt2[blk][:])

        mlp_phase(0)
        pns0 = softmax_part(0)
        # block 1 MLP with block 0's final reduction interleaved in 8-op
        # chunks so both phases pipeline across engines
        mlp_phase(1, lambda g16: reduce_rows(0, pns0, range(8 * g16, 8 * g16 + 8)))
        out_part(0)
        pns1 = softmax_part(1)
        reduce_rows(1, pns1, range(128))
        out_part(1)

    nc.compile()
    return nc


def _prep_inputs(query, keys, mask, w1, b1, prelu_a, w2, b2):
    """Host-side restaging of the full inputs into per-core DMA-friendly
    layouts. Returns list of per-core input maps."""
    query = np.asarray(query, dtype=np.float32)
    keys = np.asarray(keys, dtype=np.float32)
    mask = np.asarray(mask)
    w1 = np.asarray(w1, dtype=np.float32)
    b1 = np.asarray(b1, dtype=np.float32)
    w2 = np.asarray(w2, dtype=np.float32)
    b2 = np.asarray(b2, dtype=np.float32)
    alpha = float(np.asarray(prelu_a))
    assert abs(alpha - 0.25) < 1e-9, "kernel hardcodes PReLU slope 0.25"

    Wq, Wk, Wc, Wd = w1[:, :D], w1[:, D : 2 * D], w1[:, 2 * D : 3 * D], w1[:, 3 * D :]
    wa = np.ascontiguousarray((Wk - Wc).T).astype(BF16)         # [j, d]
    wd = np.ascontiguousarray(Wd.T).astype(BF16)                # [j, d]
    bias = (query @ (Wq + Wc).T + b1).astype(BF16)              # [B, D]
    w2c = np.ascontiguousarray(w2[:, 0:1]).astype(BF16)         # [D, 1]
    eye = np.zeros((128, 64), dtype=BF16)
    for r in range(64):
        eye[r, r] = 1
        eye[64 + r, r] = 1

    keys_bf = keys.astype(BF16)                                  # [B, T, D]
    mfull = mask.astype(BF16)
    query_bf = query.astype(BF16)

    # processing-order permutation within each block
    order = np.concatenate(
        [blk * 128 + np.asarray(LBSEQ) for blk in range(NBLK)]
    )

    in_maps = []
    for c in range(NCORES):
        s = slice(c * BC, (c + 1) * BC)
        k8 = np.ascontiguousarray(keys_bf[s][order]).reshape(NGRP, 8 * T, D)
        qtv = np.ascontiguousarray(
            query_bf[s][order].reshape(NBLK, 128, D).transpose(0, 2, 1)
        )                                                        # [NBLK, D, 128]
        btv = np.ascontiguousarray(
            bias[s][order].reshape(NBLK, 128, D).transpose(0, 2, 1)
        )                                                        # [NBLK, D, 128]
        in_maps.append(
            {
                "k8": k8,
                "mf": np.ascontiguousarray(mfull[s]),
                "qt": qtv,
                "bt": btv,
                "wa": wa,
                "wd": wd,
                "w2c": w2c,
                "eye": eye,
            }
        )
    return in_maps


def _get_module():
    if "module" not in _CACHE:
        _CACHE["module"] = _build_module()
    return _CACHE["module"]


def kernel(query, keys, mask, w1, b1, prelu_a, w2, b2):
    from concourse.bass_utils import run_bass_kernel_spmd

    nc = _get_module()
    in_maps = _prep_inputs(query, keys, mask, w1, b1, prelu_a, w2, b2)
    res = run_bass_kernel_spmd(nc, in_maps, list(range(NCORES)))
    _CACHE["last_results"] = res
    # per-core out: [NBLK, D, 128] -> [BC, D]
    outs = []
    for r in res.results:
        o = r["out"]  # [NBLK, D, 128]
        outs.append(o.transpose(0, 2, 1).reshape(BC, D))
    out = np.concatenate(outs, axis=0)
    return out.astype(np.float32)
